# revision 8
# baseline (speedup 1.0000x reference)
"""Trainium2 Bass kernel for nn_CrossAttention (B=4, NQ=NK=1024, D=1024, H=16).

Sharding: 8 cores = 4 batches x 2 head-groups (8 heads each). Per core:
  - inputs arrive pre-transposed/sliced on host (free): xqT/xkT/xvT [D, T] fp16,
    Wq/Wk/Wv column slices [D, 512] fp16, Wo row slice [512, D] fp16.
  - Q^T/K^T projections per head-pair [128, T] (lhsT = W slice, rhs = xT),
    k-inner units so the first unit's matmuls stream behind the x DMA.
  - scores computed transposed (scoresT [Tk, Tq]) per head: K=64 contraction,
    M=128 k-tokens, N=512 q-chunk -> full-width PE streams (2x the packed-quad
    scheme). Two chunks land in one 2-bank psum tile; one ScalarE exp of
    free-size 1024 (amortizes activation overhead) writes the fp16 ex tile.
  - denominators via an augmented ones-row in V (row 64 of the PV output);
    collected by DMA, single-pass reciprocal_approx_fast (18-bit), broadcast
    across partitions by DMA (partition-stride-0 read), DVE normalize mul.
  - V projection is interleaved into the scores(0) stream as PE filler while
    ScalarE runs exp; PV(p-1) units are the filler for scores(p).
  - out-projection contracts head-pairs (K=128 tiles); fp16 partials, host
    sums the two head-group partials per batch and adds the bias.
All matmuls fp16 (1 cycle/row on PE), accumulation fp32 in PSUM.
"""
import sys

sys.path.insert(0, "/opt/trn_rl_repo")

from contextlib import ExitStack

import numpy as np

import concourse.bass as bass
import concourse.tile as tile
from concourse import bacc, mybir
from concourse.bass_utils import run_bass_kernel_spmd

F32 = mybir.dt.float32
F16 = mybir.dt.float16

B, NQ, NK, D, H, HD = 4, 1024, 1024, 1024, 16, 64
NCORES = 8
HPC = 8          # heads per core
F = HPC * HD     # 512: per-core projection width
KT = D // 128    # 8 k-tiles over D
PAIRS = HPC // 2  # 4 head pairs
TKT = NK // 128  # 8 tiles over key tokens
NCH = NQ // 512  # 2 chunks over query tokens
T = NQ


def _emit(tc):
    nc = tc.nc
    ctx = ExitStack()

    xqT = nc.dram_tensor("xqT", [D, T], F16, kind="ExternalInput").ap()
    xkT = nc.dram_tensor("xkT", [D, T], F16, kind="ExternalInput").ap()
    xvT = nc.dram_tensor("xvT", [D, T], F16, kind="ExternalInput").ap()
    wq = nc.dram_tensor("wq", [D, F], F16, kind="ExternalInput").ap()
    wk = nc.dram_tensor("wk", [D, F], F16, kind="ExternalInput").ap()
    wv = nc.dram_tensor("wv", [D, F], F16, kind="ExternalInput").ap()
    wo = nc.dram_tensor("wo", [F, D], F16, kind="ExternalInput").ap()
    out = nc.dram_tensor("out", [T, D], F16, kind="ExternalOutput").ap()

    wpool = ctx.enter_context(tc.tile_pool(name="wpool", bufs=1))
    qkv = ctx.enter_context(tc.tile_pool(name="qkv", bufs=1))
    # shared slot pool for x-stream tiles and exp tiles (same 2KB/partition;
    # x tiles die as exp tiles are born, so slots recycle)
    big = ctx.enter_context(tc.tile_pool(name="big", bufs=38))
    psum = ctx.enter_context(tc.tile_pool(name="psum", bufs=2, space="PSUM"))
    nrm = ctx.enter_context(tc.tile_pool(name="nrm", bufs=2))
    ost = ctx.enter_context(tc.tile_pool(name="ost", bufs=4))

    # ---- persistent weights ----
    wq_sb = wpool.tile([128, KT, F], F16, tag="wq")
    wk_sb = wpool.tile([128, KT, F], F16, tag="wk")
    wv_sb = wpool.tile([128, KT, F], F16, tag="wv")
    wo_sb = wpool.tile([128, PAIRS, D], F16, tag="wo")

    # ---- persistent intermediates ----
    # vp padded to 128 columns (V | ones | zeros) so the PV ldweights gets
    # the compiler's fast-weight-load path (needs a full 128-column weight)
    qt = [qkv.tile([128, T], F16, tag=f"qt{p}", name=f"qt{p}") for p in range(PAIRS)]
    kt = [qkv.tile([128, T], F16, tag=f"kt{p}", name=f"kt{p}") for p in range(PAIRS)]
    vp_sb = qkv.tile([128, TKT, HPC, 128], F16, tag="vp")
    att = [qkv.tile([128, T], F16, tag=f"att{p}", name=f"att{p}") for p in range(PAIRS)]

    # ---- DMA issue order is priority order: xq gates PE start, then xk,
    # then xv; x split over sync+gpsimd queues, weights on scalar ----
    xq_t, xk_t, xv_t = [], [], []
    for name, xs, lst in (("xq", xqT, xq_t), ("xk", xkT, xk_t)):
        for k in range(KT):
            t = big.tile([128, T], F16, tag="big", name=f"{name}{k}")
            eng = nc.sync if k % 2 == 0 else nc.gpsimd
            eng.dma_start(out=t[:], in_=xs[k * 128:(k + 1) * 128, :])
            lst.append(t)
        if name == "xq":
            for k in range(KT):
                nc.scalar.dma_start(out=wq_sb[:, k, :],
                                    in_=wq[k * 128:(k + 1) * 128, :])
    for k in range(KT):
        t = big.tile([128, T], F16, tag="big", name=f"xv{k}")
        eng = nc.sync if k % 2 == 0 else nc.gpsimd
        eng.dma_start(out=t[:], in_=xvT[k * 128:(k + 1) * 128, :])
        xv_t.append(t)
    for k in range(KT):
        nc.scalar.dma_start(out=wk_sb[:, k, :], in_=wk[k * 128:(k + 1) * 128, :])
    for k in range(KT):
        nc.scalar.dma_start(out=wv_sb[:, k, :], in_=wv[k * 128:(k + 1) * 128, :])
    for p in range(PAIRS):
        nc.scalar.dma_start(out=wo_sb[:, p, :], in_=wo[p * 128:(p + 1) * 128, :])
    nc.gpsimd.memset(vp_sb[:, :, :, HD:128], 0.0)
    nc.gpsimd.memset(vp_sb[:, :, :, HD:HD + 1], 1.0)

    # preload the exp activation table while PE does projections
    warm = nrm.tile([1, 16], F32, tag="warm", bufs=1)
    nc.vector.memset(warm[:], 0.0)
    nc.scalar.activation(out=warm[:], in_=warm[:],
                         func=mybir.ActivationFunctionType.Exp)

    # ---- Q then K projections: units (m, n), k-inner; unit 0 streams
    # behind the x DMA; copies double-buffer against the other psum slot ----
    for x_t, w_sb, dst in ((xq_t, wq_sb, qt), (xk_t, wk_sb, kt)):
        for m in range(PAIRS):
            for n in range(NCH):
                ps = psum.tile([128, 512], F32, tag="pp", name=f"ps_{m}_{n}")
                for k in range(KT):
                    nc.tensor.matmul(out=ps[:],
                                     lhsT=w_sb[:, k, m * 128:(m + 1) * 128],
                                     rhs=x_t[k][:, n * 512:(n + 1) * 512],
                                     start=(k == 0), stop=(k == KT - 1))
                nc.vector.tensor_copy(out=dst[m][:, n * 512:(n + 1) * 512],
                                      in_=ps[:])

    scale = 1.0 / float(np.sqrt(HD))
    ex = {}

    def score_unit(p, hh, tkm):
        """One head's scoresT strip: [128 k-tok, 1024 q] + exp (free-1024)."""
        r0 = hh * 64
        sc = psum.tile([128, NCH, 512], F32, tag="sc",
                       name=f"sc_{p}_{hh}_{tkm}")
        for n in range(NCH):
            nc.tensor.matmul(
                out=sc[:, n, :],
                lhsT=kt[p][r0:r0 + 64, tkm * 128:(tkm + 1) * 128],
                rhs=qt[p][r0:r0 + 64, n * 512:(n + 1) * 512],
                start=True, stop=True)
        ex_t = big.tile([128, T], F16, tag="big", name=f"ex_{p}_{hh}_{tkm}")
        nc.scalar.activation(
            out=ex_t[:].rearrange("p (a b) -> p a b", a=NCH),
            in_=sc[:],
            func=mybir.ActivationFunctionType.Exp, scale=scale)
        ex[(p, hh, tkm)] = ex_t

    def v_unit(tt):
        """V projection for token-tile tt -> vp_sb[:, tt, :, 0:HD]."""
        ps = psum.tile([128, 512], F32, tag="pp", name=f"psv_{tt}")
        for k in range(KT):
            nc.tensor.matmul(out=ps[:],
                             lhsT=xv_t[k][:, tt * 128:(tt + 1) * 128],
                             rhs=wv_sb[:, k, :],
                             start=(k == 0), stop=(k == KT - 1))
        nc.vector.tensor_copy(
            out=vp_sb[:, tt, :, 0:HD],
            in_=ps[:].rearrange("p (h d) -> p h d", h=HPC))

    def norm(g, u0, u1):
        """den -> 1/den -> broadcast -> normalize head g into att[g//2]."""
        p, hh = divmod(g, 2)
        den = nrm.tile([1, T], F32, tag="den", name=f"den_{g}")
        nc.vector.tensor_copy(out=den[:, 0:512], in_=u0[64:65, :])
        nc.vector.tensor_copy(out=den[:, 512:1024], in_=u1[64:65, :])
        rec = nrm.tile([1, T], F32, tag="rec", name=f"rec_{g}")
        nc.vector.reciprocal_approx_fast(out=rec[:], in_=den[:])
        rb = nrm.tile([64, T], F32, tag="rb", name=f"rb_{g}")
        nc.gpsimd.partition_broadcast(out_ap=rb[:], in_ap=rec[:], channels=64)
        if hh == 0:
            for n, u in ((0, u0), (1, u1)):
                nc.vector.tensor_mul(out=att[p][0:64, n * 512:(n + 1) * 512],
                                     in0=u[0:64, :],
                                     in1=rb[:, n * 512:(n + 1) * 512])
        else:
            tmp = nrm.tile([64, T], F16, tag="tmp", name=f"tmp_{p}")
            for n, u in ((0, u0), (1, u1)):
                nc.vector.tensor_mul(out=tmp[:, n * 512:(n + 1) * 512],
                                     in0=u[0:64, :],
                                     in1=rb[:, n * 512:(n + 1) * 512])
            nc.gpsimd.dma_start(out=att[p][64:128, :], in_=tmp[:])

    # ---- attention: one block per head g. Block g emits scores(g) per
    # k-tile; the PE filler between score units is the k-aligned PV matmul
    # of head g-1 (whose exp tiles landed last block) plus a share of the
    # V projection in the first blocks. ScalarE streams exp back-to-back;
    # PE stays ~1 unit ahead. ----
    # all V tiles a PV matmul in block g can touch must already be emitted:
    # block 1's PV (head 0) reads v-tile k at its tkm=k slot, so v6/v7 land
    # at the top of block 1, ahead of PV k=6/7
    V_SHARE = {0: [0, 1, 2, 3, 4, 5], 1: [6, 7]}
    pv_prev = None
    for g in range(HPC + 1):
        pv_cur = None
        if g < HPC:
            p, hh = divmod(g, 2)
            pv_cur = [psum.tile([128, 512], F32, tag="pv", name=f"pv_{g}_{n}")
                      for n in range(NCH)]
        for tkm in range(TKT):
            if g < HPC:
                score_unit(p, hh, tkm)
            if pv_prev is not None:
                gp, (u0, u1) = pv_prev
                pp_, ph = divmod(gp, 2)
                for n, u in ((0, u0), (1, u1)):
                    nc.tensor.matmul(out=u[:],
                                     lhsT=vp_sb[:, tkm, gp, :],
                                     rhs=ex[(pp_, ph, tkm)][:, n * 512:(n + 1) * 512],
                                     start=(tkm == 0), stop=(tkm == TKT - 1))
            if g < HPC and tkm < len(V_SHARE.get(g, [])):
                v_unit(V_SHARE[g][tkm])
        if pv_prev is not None:
            gp, (u0, u1) = pv_prev
            norm(gp, u0, u1)
        pv_prev = (g, pv_cur) if pv_cur is not None else None

    # ---- output projection ----
    for q in range(T // 128):
        for n in range(NCH):
            po = psum.tile([128, 512], F32, tag="pp", name=f"po_{q}_{n}")
            for p4 in range(PAIRS):
                nc.tensor.matmul(out=po[:],
                                 lhsT=att[p4][:, q * 128:(q + 1) * 128],
                                 rhs=wo_sb[:, p4, n * 512:(n + 1) * 512],
                                 start=(p4 == 0), stop=(p4 == PAIRS - 1))
            ot = ost.tile([128, 512], F16, tag="ot", name=f"ot_{q}_{n}")
            if (q + n) % 2 == 0:
                nc.scalar.copy(out=ot[:], in_=po[:])
            else:
                nc.vector.tensor_copy(out=ot[:], in_=po[:])
            eng = nc.sync if (q + n) % 2 == 0 else nc.gpsimd
            eng.dma_start(out=out[q * 128:(q + 1) * 128,
                                  n * 512:(n + 1) * 512], in_=ot[:])
    ctx.close()


_NC_CACHE = None


def build():
    global _NC_CACHE
    if _NC_CACHE is None:
        nc = bacc.Bacc("TRN2", target_bir_lowering=False, debug=False,
                       num_devices=NCORES)
        with tile.TileContext(nc) as tc:
            _emit(tc)
        nc.compile()
        _NC_CACHE = nc
    return _NC_CACHE


def make_in_maps(inputs):
    q = np.asarray(inputs["query_tokens"], dtype=np.float32)
    kk = np.asarray(inputs["key_tokens"], dtype=np.float32)
    v = np.asarray(inputs["value_tokens"], dtype=np.float32)
    Wq = np.asarray(inputs["Wq"], dtype=np.float32)
    Wk = np.asarray(inputs["Wk"], dtype=np.float32)
    Wv = np.asarray(inputs["Wv"], dtype=np.float32)
    Wo = np.asarray(inputs["Wo"], dtype=np.float32)

    qT = [np.ascontiguousarray(q[b].T).astype(np.float16) for b in range(B)]
    kT = [np.ascontiguousarray(kk[b].T).astype(np.float16) for b in range(B)]
    vT = [np.ascontiguousarray(v[b].T).astype(np.float16) for b in range(B)]
    wq_g = [np.ascontiguousarray(Wq[:, g * F:(g + 1) * F]).astype(np.float16)
            for g in range(2)]
    wk_g = [np.ascontiguousarray(Wk[:, g * F:(g + 1) * F]).astype(np.float16)
            for g in range(2)]
    wv_g = [np.ascontiguousarray(Wv[:, g * F:(g + 1) * F]).astype(np.float16)
            for g in range(2)]
    wo_g = [np.ascontiguousarray(Wo[g * F:(g + 1) * F, :]).astype(np.float16)
            for g in range(2)]

    in_maps = []
    for c in range(NCORES):
        b, g = c // 2, c % 2
        in_maps.append({
            "xqT": qT[b], "xkT": kT[b], "xvT": vT[b],
            "wq": wq_g[g], "wk": wk_g[g], "wv": wv_g[g], "wo": wo_g[g],
        })
    return in_maps


def combine(results, bo):
    out = np.zeros((B, NQ, D), dtype=np.float32)
    for c in range(NCORES):
        out[c // 2] += np.asarray(results[c]["out"], dtype=np.float32)
    out += np.asarray(bo, dtype=np.float32)[None, None, :]
    return out


def kernel(**inputs):
    nc = build()
    in_maps = make_in_maps(inputs)
    res = run_bass_kernel_spmd(nc, in_maps, list(range(NCORES)))
    return combine(res.results, inputs["bo"])
